# revision 1
# baseline (speedup 1.0000x reference)
"""Trainium2 Bass kernel for nn_Contrastive_Loss (bs=8192, hidden=2048, 8 cores).

Math: reference(X, Y) = cl(X,Y) + cl(Y,X) where
  cl(A,B)[i] = -log(E_ii / (colsum_i(E) - E_ii)),  E = exp(norm(A) @ norm(B).T)
Since norm(Y)@norm(X).T = S.T, the second term's column sums are the first
term's row sums and the diagonals coincide.  With E = exp(S):
  out[i] = log(rowsum_i(E) - E_ii) + log(colsum_i(E) - E_ii) - 2*S_ii

Sharding: rows of X/Y split across 8 cores (1024 rows each).  Each core:
 - receives its raw X shard twice: row-major XS (for row norms + diagonal) and
   pre-transposed XT (hidden-major, the matmul stationary operand; the
   normalization scale 1/||x_i|| is folded into the exp activation's
   per-partition scale, so the matmul can consume raw X),
 - normalizes its Y shard to bf16, AllGathers normalized Y,
 - computes its 1024x8192 block of S via PE matmuls (K=2048), with exp+rowsum
   fused on ACT and column partial sums via ones-vector matmuls on PE,
 - ReduceScatter sums the column partials so each core gets exactly the 1024
   column sums matching its own rows.
"""

import numpy as np
from contextlib import ExitStack

import concourse.bass as bass
import concourse.bacc as bacc
import concourse.mybir as mybir
import concourse.tile as tile
from concourse.bass_utils import run_bass_kernel_spmd

FP32 = mybir.dt.float32
BF16 = mybir.dt.bfloat16

BS = 8192      # batch (rows of X and Y)
H = 2048       # hidden
NCORES = 8
RPC = BS // NCORES   # rows per core = 1024
EPS = 1e-8


def build(bs=BS, h=H, ncores=NCORES):
    rpc = bs // ncores
    mt_n = rpc // 128    # m-tiles per core
    ntw = min(512, bs)   # n-tile width (PSUM bank)
    nnt = bs // ntw      # n-tiles
    kt_n = h // 128      # k-tiles
    groups = [list(range(ncores))]

    nc = bacc.Bacc("TRN2", target_bir_lowering=False, num_devices=ncores)
    XS = nc.dram_tensor("XS", [rpc, h], FP32, kind="ExternalInput")
    XT = nc.dram_tensor("XT", [h, rpc], FP32, kind="ExternalInput")
    YS = nc.dram_tensor("YS", [rpc, h], FP32, kind="ExternalInput")
    OUT = nc.dram_tensor("OUT", [rpc, 1], FP32, kind="ExternalOutput")

    with tile.TileContext(nc) as tc, ExitStack() as ctx:
        dram = ctx.enter_context(tc.tile_pool(name="dram", bufs=1, space="DRAM"))
        prep = ctx.enter_context(tc.tile_pool(name="prep", bufs=3))
        junkp = ctx.enter_context(tc.tile_pool(name="junkp", bufs=2))
        small = ctx.enter_context(tc.tile_pool(name="small", bufs=4))
        stat = ctx.enter_context(tc.tile_pool(name="stat", bufs=1))
        wpool = ctx.enter_context(tc.tile_pool(name="wpool", bufs=1))
        ypool = ctx.enter_context(tc.tile_pool(name="ypool", bufs=3))
        epool = ctx.enter_context(tc.tile_pool(name="epool", bufs=18))
        cpool = ctx.enter_context(tc.tile_pool(name="cpool", bufs=2))
        psum = ctx.enter_context(tc.tile_pool(name="psum", bufs=4, space="PSUM"))
        cpsum = ctx.enter_context(tc.tile_pool(name="cpsum", bufs=2, space="PSUM"))

        YnD = dram.tile([rpc, h], BF16)
        # AllGather in 4 chunks so matmuls start before the full gather lands.
        # Chunk j gathers every rank's local Y rows [j*cw, (j+1)*cw).
        nch = 4 if mt_n % 4 == 0 else 1
        cw = rpc // nch                      # rows per chunk per rank
        YnC = [
            dram.tile([ncores * cw, h], BF16, addr_space="Shared",
                      name=f"YnC{j}", tag=f"YnC{j}")
            for j in range(nch)
        ]
        CS = dram.tile([bs], FP32)
        CSR = dram.tile([rpc], FP32)

        # persistent stats (p = row % 128, column m = row // 128)
        sdiag = stat.tile([128, mt_n], FP32)        # S_ii
        invx = stat.tile([128, mt_n], FP32)         # 1/||x_i||
        rstot = stat.tile([128, mt_n], FP32)        # rowsum(E)
        rsacc = stat.tile([128, mt_n * nnt], FP32)  # per-(m,nt) rowsums

        # ---- raw X^T -> SBUF as bf16 (cast in SWDGE dma) ----
        # xnt[p, k, m] = X[shard_row m, 128k+p]
        xnt = wpool.tile([128, kt_n, rpc], BF16)
        nc.gpsimd.dma_start(
            out=xnt[:], in_=XT.rearrange("(k p) m -> p k m", p=128)
        )

        # ---------------- Phase A: norms, normalized Y, diagonal ----------------
        def row_norm_inv(t, tag):
            """per-row 1/max(||row||, eps) for a [128, h] tile"""
            junk = junkp.tile([128, h], BF16, tag="junk", name="junk")
            ss = small.tile([128, 1], FP32, tag="ss", name="ss")
            nc.scalar.activation(
                junk[:], t[:], mybir.ActivationFunctionType.Square, accum_out=ss[:]
            )
            nrm = small.tile([128, 1], FP32, tag="nrm", name="nrm")
            nc.scalar.sqrt(nrm[:], ss[:])
            nc.vector.tensor_scalar_max(nrm[:], nrm[:], EPS)
            inv = small.tile([128, 1], FP32, tag="inv", name="inv")
            nc.vector.reciprocal(inv[:], nrm[:])
            return inv

        for m in range(mt_n):
            r0 = m * 128
            ys = prep.tile([128, h], FP32, tag="ldy", name="ldy")
            nc.gpsimd.dma_start(out=ys[:], in_=YS[r0 : r0 + 128, :])
            iy = row_norm_inv(ys, "y")
            yn = prep.tile([128, h], BF16, tag="yn", name="yn")
            nc.scalar.mul(yn[:], ys[:], iy[:])
            nc.gpsimd.dma_start(out=YnD[r0 : r0 + 128, :], in_=yn[:])

            xs = prep.tile([128, h], FP32, tag="ldx", name="ldx")
            nc.gpsimd.dma_start(out=xs[:], in_=XS[r0 : r0 + 128, :])
            ix = row_norm_inv(xs, "x")
            nc.vector.tensor_copy(invx[:, m : m + 1], ix[:])

            # diagonal: S_ii = (x_i . yn_i) / ||x_i||
            prod = junkp.tile([128, h], FP32, tag="prod", name="prod")
            nc.vector.tensor_mul(prod[:], xs[:], yn[:])
            sdr = small.tile([128, 1], FP32, tag="sdr", name="sdr")
            nc.vector.reduce_sum(sdr[:], prod[:], axis=mybir.AxisListType.X)
            nc.vector.tensor_mul(sdiag[:, m : m + 1], sdr[:], ix[:])

            # once this AG chunk's Y rows are staged, gather them
            if (m + 1) % (cw // 128) == 0:
                j = m // (cw // 128)
                nc.gpsimd.collective_compute(
                    "AllGather", mybir.AluOpType.bypass, replica_groups=groups,
                    ins=[YnD[j * cw : (j + 1) * cw, :]], outs=[YnC[j].opt()],
                )

        ones = stat.tile([128, 1], BF16)
        nc.vector.memset(ones[:], 1.0)

        # ---------------- Phase B: S block, exp, row/col sums ----------------
        # The ones-matmul column reduction of n-tile `nt` is interleaved into
        # n-tile `nt+1`'s S-matmul stream: by then every E tile of `nt` is
        # ready, so PE never stalls waiting on ACT's exp.
        def flush_colsum(prev_state, m):
            pnt, pcolps, pets = prev_state
            nc.tensor.matmul(
                pcolps[:], lhsT=ones[:], rhs=pets[m][:],
                start=(m == 0), stop=(m == mt_n - 1),
            )
            if m == mt_n - 1:
                cssb = cpool.tile([1, ntw], FP32, tag="cssb", name="cssb")
                nc.vector.tensor_copy(cssb[:], pcolps[:])
                nc.gpsimd.dma_start(
                    out=CS[pnt * ntw : (pnt + 1) * ntw], in_=cssb[:]
                )

        def tile_chunks(nt):
            """(chunk j, src row in YnC[j], nrows, dest offset) for n-tile nt"""
            if nch == 1:
                return [(0, nt * ntw, ntw, 0)]
            r = (nt * ntw) // rpc
            l0 = nt * ntw - r * rpc
            out = []
            for j in range(l0 // cw, (l0 + ntw - 1) // cw + 1):
                lo, hi = max(l0, j * cw), min(l0 + ntw, (j + 1) * cw)
                out.append((j, r * cw + lo - j * cw, hi - lo, lo - l0))
            return out

        # process n-tiles needing early AG chunks first
        nt_order = sorted(
            range(nnt), key=lambda nt: (max(c[0] for c in tile_chunks(nt)), nt)
        )
        prev = None
        for nt in nt_order:
            # ynt[p, k, n] = Yn[nt*ntw + n, 128k+p]  (xbar transpose DMAs)
            ynt = ypool.tile([128, kt_n, ntw], BF16, tag="ynt", name="ynt")
            for (j, srow, nrows, off) in tile_chunks(nt):
                nc.sync.dma_start(
                    out=ynt[:, :, off : off + nrows],
                    in_=YnC[j][srow : srow + nrows, :],
                    transpose=True,
                )
            colps = cpsum.tile([1, ntw], FP32, tag="colps", name="colps")
            ets = []
            for m in range(mt_n):
                ps = psum.tile([128, ntw], FP32, tag="S", name="S")
                for kt in range(kt_n):
                    nc.tensor.matmul(
                        ps[:],
                        lhsT=xnt[:, kt, m * 128 : (m + 1) * 128],
                        rhs=ynt[:, kt, :],
                        start=(kt == 0),
                        stop=(kt == kt_n - 1),
                    )
                et = epool.tile([128, ntw], BF16, tag="E", name="E")
                nc.scalar.activation(
                    et[:], ps[:], mybir.ActivationFunctionType.Exp,
                    scale=invx[:, m : m + 1],
                    accum_out=rsacc[:, m * nnt + nt : m * nnt + nt + 1],
                )
                ets.append(et)
                if prev is not None:
                    flush_colsum(prev, m)
            prev = (nt, colps, ets)
        for m in range(mt_n):
            flush_colsum(prev, m)

        # ---------------- ReduceScatter column sums ----------------
        nc.gpsimd.collective_compute(
            "ReduceScatter", mybir.AluOpType.add, replica_groups=groups,
            ins=[CS.opt()], outs=[CSR.opt()],
        )

        # ---------------- Finale ----------------
        csr = stat.tile([128, mt_n], FP32)
        nc.gpsimd.dma_start(out=csr[:], in_=CSR.rearrange("(a b) -> b a", b=128))
        for m in range(mt_n):
            nc.vector.reduce_sum(
                rstot[:, m : m + 1], rsacc[:, m * nnt : (m + 1) * nnt],
                axis=mybir.AxisListType.X,
            )
        edig = stat.tile([128, mt_n], FP32)
        nc.scalar.activation(edig[:], sdiag[:], mybir.ActivationFunctionType.Exp)
        negr = stat.tile([128, mt_n], FP32)
        nc.vector.tensor_sub(negr[:], rstot[:], edig[:])
        negc = stat.tile([128, mt_n], FP32)
        nc.vector.tensor_sub(negc[:], csr[:], edig[:])
        lr = stat.tile([128, mt_n], FP32)
        nc.scalar.activation(lr[:], negr[:], mybir.ActivationFunctionType.Ln)
        lcv = stat.tile([128, mt_n], FP32)
        nc.scalar.activation(lcv[:], negc[:], mybir.ActivationFunctionType.Ln)
        res = stat.tile([128, mt_n], FP32)
        nc.vector.tensor_add(res[:], lr[:], lcv[:])
        d2 = stat.tile([128, mt_n], FP32)
        nc.vector.tensor_scalar_mul(d2[:], sdiag[:], -2.0)
        nc.vector.tensor_add(res[:], res[:], d2[:])
        nc.gpsimd.dma_start(
            out=OUT.rearrange("(a b) c -> b (a c)", b=128), in_=res[:]
        )

    nc.compile()
    return nc


_CACHE = {}


def _get_nc():
    if "nc" not in _CACHE:
        _CACHE["nc"] = build()
    return _CACHE["nc"]


def make_in_maps(X, Y, ncores=NCORES, rpc=RPC):
    maps = []
    for i in range(ncores):
        xs = np.ascontiguousarray(X[i * rpc : (i + 1) * rpc])
        maps.append({
            "XS": xs,
            "XT": np.ascontiguousarray(xs.T),
            "YS": np.ascontiguousarray(Y[i * rpc : (i + 1) * rpc]),
        })
    return maps


def kernel(X, Y):
    X = np.ascontiguousarray(np.asarray(X, dtype=np.float32))
    Y = np.ascontiguousarray(np.asarray(Y, dtype=np.float32))
    assert X.shape == (BS, H) and Y.shape == (BS, H)
    nc = _get_nc()
    r = run_bass_kernel_spmd(nc, make_in_maps(X, Y), list(range(NCORES)))
    out = np.concatenate([r.results[i]["OUT"] for i in range(NCORES)], axis=0)
    return out.astype(np.float32)

